# revision 1
# baseline (speedup 1.0000x reference)
"""Trainium2 Bass kernel for nn_EruSelfAttentionModel (B=4,S=1024,E=1024,A=64,H=16,L=2).

Sharding: 8 cores; core c handles batch c//2 and heads (c%2)*8..(c%2)*8+8.
Heads are fully independent through both layers, so each core runs its
(batch, 8-head) slice end-to-end with no collectives.

Per-core dataflow (everything in transposed [E,S] activation layout so the
attention chain needs no per-head transposes):
  stage0: indirect-DMA embedding gather -> LayerNorm (bn_stats, [S,E] layout)
          -> PE transpose -> hn0T [E,S] in SBUF (shared by all 8 heads' L1)
  per head, layer 1: qkT / v / scoresT / exp / z (ones-matmul) / outT -> DRAM ht1[h]
  per head, layer 2: LN via PE ones-matmul stats (in [E,S] layout) -> attention,
          final out emitted in [S,E] layout directly (lhsT/rhs swap) -> output.
"""

import math
import os
from contextlib import ExitStack

import numpy as np

# The device path (bass2jax under axon) needs the axon PJRT backend; a
# JAX_PLATFORMS=cpu pin (common for running the jax reference) would break it.
if "JAX_PLATFORMS" in os.environ and "axon" not in os.environ["JAX_PLATFORMS"]:
    del os.environ["JAX_PLATFORMS"]

import bass_rust
from bass_rust import SyncInfo
import concourse.bass as bass
import concourse.mybir as mybir
import concourse.tile as tile
from concourse.bass_utils import run_bass_kernel_spmd
from concourse.masks import make_identity

B, S, E, A, H, L, V = 4, 1024, 1024, 64, 16, 2, 32000
EPS = 1e-5
SCALE = math.sqrt(E)
P = 128
KO = E // P       # 8 k-blocks over E
SB = S // P       # 8 s-blocks
NH = H // 2       # 8 heads per core
HALF = S // 2     # 512
FP = mybir.dt.float32
AF = mybir.ActivationFunctionType
OP = mybir.AluOpType

_WID = [0]


def _legalize_multi_waits(nc, max_keep=1):
    """This walrus build accepts at most one sync-wait command per engine
    instruction; split extras into standalone EventSemaphore waits."""
    for f in nc.m.functions:
        for blk in f.blocks:
            out = []
            changed = False
            for inst in blk.instructions:
                si = inst.sync_info
                ow = list(si.on_wait) if si is not None else []
                if len(ow) > max_keep:
                    changed = True
                    for w in ow[:-max_keep]:
                        _WID[0] += 1
                        out.append(bass_rust.InstEventSemaphore(
                            name=f"WSPLIT-{_WID[0]}",
                            engine=inst.engine,
                            ins=[], outs=[],
                            sync_info=SyncInfo(on_wait=[w], on_update=[]),
                        ))
                    inst.sync_info = SyncInfo(on_wait=ow[-max_keep:],
                                              on_update=list(si.on_update))
                out.append(inst)
            if changed:
                blk.instructions = out


def _bcast_mid(ap, n):
    """[P, F] -> [P, n, F] with stride-0 middle dim (free-dim broadcast)."""
    return ap[:, None, :].to_broadcast([ap.shape[0], n, ap.shape[1]])


def _build_nc(g0_identity, g1_identity, legalize=True):
    nc = bass.Bass("TRN2")

    emb = nc.dram_tensor("emb", [V, E], FP, kind="ExternalInput")
    xidx = nc.dram_tensor("xidx", [S, 1], mybir.dt.int32, kind="ExternalInput")
    wqk = nc.dram_tensor("wqk", [L, NH, E, 2 * A], FP, kind="ExternalInput")
    wv = nc.dram_tensor("wv", [L, NH, E, E], FP, kind="ExternalInput")
    g0 = nc.dram_tensor("g0", [E], FP, kind="ExternalInput")
    b0 = nc.dram_tensor("b0", [E], FP, kind="ExternalInput")
    g1c = nc.dram_tensor("g1c", [P, KO], FP, kind="ExternalInput")
    b1c = nc.dram_tensor("b1c", [P, KO], FP, kind="ExternalInput")
    out_d = nc.dram_tensor("out", [NH, S, E], FP, kind="ExternalOutput")
    ht1 = nc.dram_tensor("ht1", [NH, KO, P, S], FP)  # Internal, [E,S] layout

    with tile.TileContext(nc) as tc, ExitStack() as ctx:
        const = ctx.enter_context(tc.tile_pool(name="const", bufs=1))
        hnp = ctx.enter_context(tc.tile_pool(name="hnp", bufs=2))
        h0p = ctx.enter_context(tc.tile_pool(name="h0p", bufs=2))
        wvp = ctx.enter_context(tc.tile_pool(name="wvp", bufs=1))
        wqkp = ctx.enter_context(tc.tile_pool(name="wqkp", bufs=2))
        vp = ctx.enter_context(tc.tile_pool(name="vp", bufs=1))
        wtp = ctx.enter_context(tc.tile_pool(name="wtp", bufs=1))
        qkp = ctx.enter_context(tc.tile_pool(name="qkp", bufs=1))
        outp = ctx.enter_context(tc.tile_pool(name="outp", bufs=3))
        statp = ctx.enter_context(tc.tile_pool(name="statp", bufs=1))
        smallp = ctx.enter_context(tc.tile_pool(name="smallp", bufs=2))
        psA = ctx.enter_context(tc.tile_pool(name="psA", bufs=2, space="PSUM"))
        psS = ctx.enter_context(tc.tile_pool(name="psS", bufs=2, space="PSUM"))
        psZO = ctx.enter_context(tc.tile_pool(name="psZO", bufs=2, space="PSUM"))

        ident = const.tile([P, P], FP)
        make_identity(nc, ident[:])
        ones128 = const.tile([P, P], FP)
        nc.vector.memset(ones128[:], 1.0)
        eps_t = const.tile([P, 1], FP)
        nc.vector.memset(eps_t[:], EPS)
        oneE = const.tile([P, P], FP)
        nc.vector.memset(oneE[:], 1.0 / E)
        if not g0_identity:
            g0rep = const.tile([P, E], FP)
            b0rep = const.tile([P, E], FP)
            nc.sync.dma_start(g0rep[:], g0.ap()[None, :].to_broadcast([P, E]))
            nc.sync.dma_start(b0rep[:], b0.ap()[None, :].to_broadcast([P, E]))
        if not g1_identity:
            g1t = const.tile([P, KO], FP)
            b1t = const.tile([P, KO], FP)
            nc.sync.dma_start(g1t[:], g1c[:, :])
            nc.sync.dma_start(b1t[:], b1c[:, :])

        # ---------------- stage 0: embed + LN0 + transpose -> hn0T ----------
        hn0T = hnp.tile([P, KO, S], FP, tag="hn")  # [e_inner, e_outer, s]
        for sb in range(SB):
            h0sb = h0p.tile([P, E], FP, tag="h0sb")
            idxt = smallp.tile([P, 1], mybir.dt.int32, tag="idx")
            nc.sync.dma_start(idxt[:], xidx[sb * P:(sb + 1) * P, :])
            nc.gpsimd.indirect_dma_start(
                out=h0sb[:], out_offset=None, in_=emb[:, :],
                in_offset=bass.IndirectOffsetOnAxis(ap=idxt[:, :1], axis=0),
            )
            # LayerNorm over free dim (E) via bn_stats
            stats = smallp.tile([P, 2, 6], FP, tag="bnst")
            nc.vector.bn_stats(stats[:, 0, :], h0sb[:, 0:512])
            nc.vector.bn_stats(stats[:, 1, :], h0sb[:, 512:1024])
            mv = smallp.tile([P, 2], FP, tag="bnmv")
            nc.vector.bn_aggr(mv[:], stats[:])
            rstd = smallp.tile([P, 1], FP, tag="rstd")
            nc.scalar.activation(rstd[:], mv[:, 1:2], AF.Sqrt, bias=eps_t[:])
            nc.vector.reciprocal(rstd[:], rstd[:])
            nc.vector.tensor_scalar(h0sb[:], h0sb[:], scalar1=mv[:, 0:1],
                                    scalar2=rstd[:], op0=OP.subtract, op1=OP.mult)
            if not g0_identity:
                nc.vector.tensor_tensor(h0sb[:], h0sb[:], g0rep[:], OP.mult)
                nc.vector.tensor_tensor(h0sb[:], h0sb[:], b0rep[:], OP.add)
            for eo in range(KO):
                pst = psS.tile([P, P], FP, tag="ps_s")
                nc.tensor.transpose(pst[:], h0sb[:, eo * P:(eo + 1) * P], ident[:])
                nc.any.tensor_copy(hn0T[:, eo, sb * P:(sb + 1) * P], pst[:])

        # ---------------- attention unit ------------------------------------
        def attn_unit(layer, head, hn, final):
            wqk_sb = wqkp.tile([P, KO, 2 * A], FP, tag="wqk")
            nc.sync.dma_start(wqk_sb[:],
                              wqk.ap()[layer, head].rearrange("(ko p) m -> p ko m", p=P))
            wv_sb = wvp.tile([P, KO, E], FP, tag="wv")
            nc.sync.dma_start(wv_sb[:],
                              wv.ap()[layer, head].rearrange("(ko p) o -> p ko o", p=P))

            # qkT: [2A=128, S] packed q (rows 0:64) and k (rows 64:128)
            ps_qk = psA.tile([P, S], FP, tag="big")
            for nb in range(2):
                for ko in range(KO):
                    nc.tensor.matmul(ps_qk[:, nb * 512:(nb + 1) * 512],
                                     lhsT=wqk_sb[:, ko, :],
                                     rhs=hn[:, ko, nb * 512:(nb + 1) * 512],
                                     start=(ko == 0), stop=(ko == KO - 1))
            qT = qkp.tile([A, S], FP, tag="qT")
            kT = qkp.tile([A, S], FP, tag="kT")
            nc.any.tensor_copy(qT[:], ps_qk[0:A, :])
            nc.any.tensor_copy(kT[:], ps_qk[A:2 * A, :])

            # v: [T, O] (t on partitions)
            v_sb = vp.tile([P, SB, E], FP, tag="v")
            for tb in range(SB):
                ps_v = psA.tile([P, E], FP, tag="big")
                for nb in range(2):
                    for ko in range(KO):
                        nc.tensor.matmul(ps_v[:, nb * 512:(nb + 1) * 512],
                                         lhsT=hn[:, ko, tb * P:(tb + 1) * P],
                                         rhs=wv_sb[:, ko, nb * 512:(nb + 1) * 512],
                                         start=(ko == 0), stop=(ko == KO - 1))
                nc.any.tensor_copy(v_sb[:, tb, :], ps_v[:])

            for sh in range(2):
                s0 = sh * HALF
                # scoresT + exp -> wT [T, s-half]
                wT = wtp.tile([P, SB, HALF], FP, tag="wt")
                for tb in range(SB):
                    ps_s = psS.tile([P, HALF], FP, tag="ps_s")
                    nc.tensor.matmul(ps_s[:], lhsT=kT[:, tb * P:(tb + 1) * P],
                                     rhs=qT[:, s0:s0 + HALF], start=True, stop=True)
                    nc.scalar.activation(wT[:, tb, :], ps_s[:], AF.Exp,
                                         scale=float(1.0 / SCALE))
                # z = sum_t exp: partial tile-sum on DVE (8 -> 1), then one
                # K=128 ones-matmul for the cross-partition reduction
                # (replicated across partitions).
                zsum = smallp.tile([P, HALF], FP, tag="zsum")
                nc.vector.tensor_tensor(zsum[:], wT[:, 0, :], wT[:, 1, :], OP.add)
                for tb in range(2, SB):
                    nc.vector.tensor_tensor(zsum[:], zsum[:], wT[:, tb, :], OP.add)
                ps_z = psZO.tile([P, HALF], FP, tag="zo")
                nc.tensor.matmul(ps_z[:], lhsT=ones128[:], rhs=zsum[:],
                                 start=True, stop=True)
                invz = smallp.tile([P, HALF], FP, tag="invz")
                nc.vector.reciprocal(invz[:], ps_z[:])

                if not final:
                    # outT [O, s-half] -> DRAM ht1[head] ([E,S] layout)
                    for ob in range(KO):
                        ps_o = psZO.tile([P, HALF], FP, tag="zo")
                        for tb in range(SB):
                            nc.tensor.matmul(ps_o[:],
                                             lhsT=v_sb[:, tb, ob * P:(ob + 1) * P],
                                             rhs=wT[:, tb, :],
                                             start=(tb == 0), stop=(tb == SB - 1))
                        ot = outp.tile([P, HALF], FP, tag="ot")
                        nc.vector.tensor_tensor(ot[:], ps_o[:], invz[:], OP.mult)
                        nc.sync.dma_start(ht1.ap()[head, ob, :, s0:s0 + HALF], ot[:])
                else:
                    # per-partition 1/z column via PE transpose of invz blocks
                    izc = smallp.tile([P, 4], FP, tag="izc")
                    for sbb in range(4):
                        pst = psS.tile([P, P], FP, tag="ps_s")
                        nc.tensor.transpose(pst[:], invz[:, sbb * P:(sbb + 1) * P],
                                            ident[:])
                        nc.any.tensor_copy(izc[:, sbb:sbb + 1], pst[:, 0:1])
                    # out [s, O] directly (lhsT = wT slice)
                    for sbb in range(4):
                        for nb in range(2):
                            ps_o = psZO.tile([P, HALF], FP, tag="zo")
                            for tb in range(SB):
                                nc.tensor.matmul(ps_o[:],
                                                 lhsT=wT[:, tb, sbb * P:(sbb + 1) * P],
                                                 rhs=v_sb[:, tb, nb * 512:(nb + 1) * 512],
                                                 start=(tb == 0), stop=(tb == SB - 1))
                            ot = outp.tile([P, HALF], FP, tag="ot")
                            nc.vector.tensor_scalar_mul(ot[:], ps_o[:],
                                                        izc[:, sbb:sbb + 1])
                            nc.sync.dma_start(
                                out_d.ap()[head, (sh * 4 + sbb) * P:(sh * 4 + sbb + 1) * P,
                                           nb * 512:(nb + 1) * 512],
                                ot[:])

        # ---------------- layer 1 (shared hn0T) ------------------------------
        for head in range(NH):
            attn_unit(0, head, hn0T, final=False)

        # ---------------- layer 2: LN (transposed stats) + attention ---------
        def l2_ln(head):
            """Load ht1[head], LayerNorm it in place ([E,S] layout, stats via
            mean-matmuls with a 1/E constant lhsT). Returns the hn tile."""
            ht = hnp.tile([P, KO, S], FP, tag="hn")
            nc.sync.dma_start(ht[:], ht1.ap()[head].rearrange("ko p s -> p ko s"))
            # partial sums over e_outer on DVE (8 tiles -> 1), then a single
            # K=128 (1/E)-matmul per half for the cross-partition reduction
            husum = statp.tile([P, S], FP, tag="husum")
            nc.vector.tensor_tensor(husum[:], ht[:, 0, :], ht[:, 1, :], OP.add)
            for ko in range(2, KO):
                nc.vector.tensor_tensor(husum[:], husum[:], ht[:, ko, :], OP.add)
            sqsum = statp.tile([P, S], FP, tag="sqsum")
            for nb in range(2):
                sl = slice(nb * 512, (nb + 1) * 512)
                nc.vector.tensor_tensor(sqsum[:, sl], ht[:, 0, sl], ht[:, 0, sl],
                                        OP.mult)
                for ko in range(1, KO):
                    sqt = smallp.tile([P, 512], FP, tag="sqt")
                    nc.vector.tensor_tensor(sqt[:], ht[:, ko, sl], ht[:, ko, sl],
                                            OP.mult)
                    nc.vector.tensor_tensor(sqsum[:, sl], sqsum[:, sl], sqt[:],
                                            OP.add)
            mu = statp.tile([P, S], FP, tag="mu")
            var = statp.tile([P, S], FP, tag="var")
            for nb in range(2):
                sl = slice(nb * 512, (nb + 1) * 512)
                mu_ps = psA.tile([P, 512], FP, tag="big")
                nc.tensor.matmul(mu_ps[:], lhsT=oneE[:], rhs=husum[:, sl],
                                 start=True, stop=True)
                nc.any.tensor_copy(mu[:, sl], mu_ps[:])
                nc.vector.tensor_tensor(var[:, sl], mu[:, sl], mu[:, sl], OP.mult)
                sq_ps = psA.tile([P, 512], FP, tag="big")
                nc.tensor.matmul(sq_ps[:], lhsT=oneE[:], rhs=sqsum[:, sl],
                                 start=True, stop=True)
                nc.vector.tensor_tensor(var[:, sl], sq_ps[:], var[:, sl],
                                        OP.subtract)
                nc.scalar.activation(var[:, sl], var[:, sl], AF.Sqrt,
                                     bias=eps_t[:])
                nc.vector.reciprocal(var[:, sl], var[:, sl])
            # normalize in place per (half, e_outer) slice: (ht - mu) * rstd.
            # Sliced so downstream qk matmuls (which consume half 0 first)
            # can start while later slices are still normalizing.
            for nb in range(2):
                sl = slice(nb * 512, (nb + 1) * 512)
                for eo in range(KO):
                    nc.vector.tensor_tensor(ht[:, eo, sl], ht[:, eo, sl],
                                            mu[:, sl], OP.subtract)
                    nc.vector.tensor_tensor(ht[:, eo, sl], ht[:, eo, sl],
                                            var[:, sl], OP.mult)
                    if not g1_identity:
                        nc.vector.tensor_scalar(ht[:, eo, sl], ht[:, eo, sl],
                                                scalar1=g1t[:, eo:eo + 1],
                                                scalar2=b1t[:, eo:eo + 1],
                                                op0=OP.mult, op1=OP.add)
            return ht

        for head in range(NH):
            hn_cur = l2_ln(head)
            attn_unit(1, head, hn_cur, final=True)

    if legalize:
        _legalize_multi_waits(nc)
    return nc


_CACHE = {}


def _get_nc(g0_identity, g1_identity, legalize=True):
    key = (g0_identity, g1_identity, legalize)
    if key not in _CACHE:
        _CACHE[key] = _build_nc(g0_identity, g1_identity, legalize)
    return _CACHE[key]


def _prep_in_maps(x, emb, ln_gamma, ln_beta, Wq, Wk, Wv):
    x = np.asarray(x)
    emb = np.ascontiguousarray(np.asarray(emb, dtype=np.float32))
    ln_gamma = np.asarray(ln_gamma, dtype=np.float32)
    ln_beta = np.asarray(ln_beta, dtype=np.float32)
    Wq = np.asarray(Wq, dtype=np.float32)
    Wk = np.asarray(Wk, dtype=np.float32)
    Wv = np.asarray(Wv, dtype=np.float32)

    # [L,H,E,2A] packed (WqT | WkT); [L,H,E,E] = WvT
    wqkT = np.concatenate([Wq.transpose(0, 1, 3, 2), Wk.transpose(0, 1, 3, 2)],
                          axis=3)
    wvT = Wv.transpose(0, 1, 3, 2)

    g1c = np.ascontiguousarray(ln_gamma[1].reshape(KO, P).T)
    b1c = np.ascontiguousarray(ln_beta[1].reshape(KO, P).T)

    in_maps = []
    for c in range(8):
        b = c // 2
        hs = (c % 2) * NH
        in_maps.append({
            "emb": emb,
            "xidx": np.ascontiguousarray(x[b].astype(np.int32).reshape(S, 1)),
            "wqk": np.ascontiguousarray(wqkT[:, hs:hs + NH]),
            "wv": np.ascontiguousarray(wvT[:, hs:hs + NH]),
            "g0": np.ascontiguousarray(ln_gamma[0]),
            "b0": np.ascontiguousarray(ln_beta[0]),
            "g1c": g1c,
            "b1c": b1c,
        })
    g0_id = bool(np.all(ln_gamma[0] == 1.0) and np.all(ln_beta[0] == 0.0))
    g1_id = bool(np.all(ln_gamma[1] == 1.0) and np.all(ln_beta[1] == 0.0))
    return in_maps, g0_id, g1_id


def run(inputs, trace=False, trace_cores=None):
    in_maps, g0_id, g1_id = _prep_in_maps(**inputs)
    nc = _get_nc(g0_id, g1_id)
    res = run_bass_kernel_spmd(nc, in_maps, core_ids=list(range(8)),
                               trace=trace, trace_cores=trace_cores)
    out = np.empty((B, H, S, E), dtype=np.float32)
    for c in range(8):
        out[c // 2, (c % 2) * NH:(c % 2) * NH + NH] = res.results[c]["out"]
    return out, res


def kernel(x, emb, ln_gamma, ln_beta, Wq, Wk, Wv):
    out, _ = run(dict(x=x, emb=emb, ln_gamma=ln_gamma, ln_beta=ln_beta,
                      Wq=Wq, Wk=Wk, Wv=Wv))
    return out



# revision 6
# speedup vs baseline: 3.3881x; 3.3881x over previous
"""Trainium2 Bass kernel for nn_EruSelfAttentionModel (B=4,S=1024,E=1024,A=64,H=16,L=2).

Sharding: 8 cores; core c handles batch c//2 and heads (c%2)*8..(c%2)*8+8.
Heads are independent through both layers, so each core runs its (batch,
8-head) slice end-to-end with no collectives.

v2 (this file): all matmuls in bf16 (4x PE throughput vs fp32), both layers
fused per head (no DRAM round-trip for the inter-layer activations), and the
layer-1 softmax division is folded into the inter-layer LayerNorm via LN's
scale invariance: LN(out/z) == LN(out_raw) with the eps bias corrected to
eps*z^2 per row (z columns obtained by PE-transposing the ones-matmul z).

Per-core dataflow per head:
  stage0 (once): indirect-DMA gather of bf16 embeddings -> LN (bn_stats) ->
      PE transpose -> hn0T [E,S] bf16 in SBUF (shared by all 8 heads).
  layer 1: qkT / scores+exp(wT bf16) / v(bf16) / z (fp32r ones-matmul) /
      out_raw [S,E] fp32 in SBUF -> LN (eps*z^2 bias) -> PE transpose ->
      hn1T [E,S] bf16.
  layer 2: same attention, final out = psum * (1/z) column, fp32 -> DRAM.
"""

import math
import os
from contextlib import ExitStack

import numpy as np
import ml_dtypes

# The device path (bass2jax under axon) needs the axon PJRT backend; a
# JAX_PLATFORMS=cpu pin (common for running the jax reference) would break it.
if "JAX_PLATFORMS" in os.environ and "axon" not in os.environ["JAX_PLATFORMS"]:
    del os.environ["JAX_PLATFORMS"]

import bass_rust
from bass_rust import SyncInfo
import concourse.bass as bass
import concourse.mybir as mybir
import concourse.tile as tile
from concourse.bass_utils import run_bass_kernel_spmd
from concourse.masks import make_identity

B, S, E, A, H, L, V = 4, 1024, 1024, 64, 16, 2, 32000
EPS = 1e-5
SCALE = math.sqrt(E)
P = 128
KO = E // P       # 8 k-blocks over E
SB = S // P       # 8 s-blocks
NH = H // 2       # 8 heads per core
HALF = S // 2     # 512
TA = 2 * A        # 128 (packed q|k)
FP = mybir.dt.float32
FR = mybir.dt.float32r
BF = mybir.dt.bfloat16
AF = mybir.ActivationFunctionType
OP = mybir.AluOpType

_WID = [0]


def _legalize_multi_waits(nc, max_keep=1):
    """This walrus build accepts at most one sync-wait command per engine
    instruction; split extras into standalone EventSemaphore waits."""
    for f in nc.m.functions:
        for blk in f.blocks:
            out = []
            changed = False
            for inst in blk.instructions:
                si = inst.sync_info
                ow = list(si.on_wait) if si is not None else []
                if len(ow) > max_keep:
                    changed = True
                    for w in ow[:-max_keep]:
                        _WID[0] += 1
                        out.append(bass_rust.InstEventSemaphore(
                            name=f"WSPLIT-{_WID[0]}",
                            engine=inst.engine,
                            ins=[], outs=[],
                            sync_info=SyncInfo(on_wait=[w], on_update=[]),
                        ))
                    inst.sync_info = SyncInfo(on_wait=ow[-max_keep:],
                                              on_update=list(si.on_update))
                out.append(inst)
            if changed:
                blk.instructions = out


def _build_nc(g0_identity, g1_identity, legalize=True):
    nc = bass.Bass("TRN2")

    emb = nc.dram_tensor("emb", [V, E], BF, kind="ExternalInput")
    xidx = nc.dram_tensor("xidx", [S, 1], mybir.dt.int32, kind="ExternalInput")
    wqk = nc.dram_tensor("wqk", [L, NH, E, TA], BF, kind="ExternalInput")
    wv = nc.dram_tensor("wv", [L, NH, E, E], BF, kind="ExternalInput")
    g0 = nc.dram_tensor("g0", [E], FP, kind="ExternalInput")
    b0 = nc.dram_tensor("b0", [E], FP, kind="ExternalInput")
    g1 = nc.dram_tensor("g1", [E], FP, kind="ExternalInput")
    b1 = nc.dram_tensor("b1", [E], FP, kind="ExternalInput")
    out_d = nc.dram_tensor("out", [NH, S, E], FP, kind="ExternalOutput")

    with tile.TileContext(nc) as tc, ExitStack() as ctx:
        const = ctx.enter_context(tc.tile_pool(name="const", bufs=1))
        hn0p = ctx.enter_context(tc.tile_pool(name="hn0p", bufs=1))
        hn1p = ctx.enter_context(tc.tile_pool(name="hn1p", bufs=1))
        wqkp = ctx.enter_context(tc.tile_pool(name="wqkp", bufs=2))
        wvp = ctx.enter_context(tc.tile_pool(name="wvp", bufs=2))
        vp = ctx.enter_context(tc.tile_pool(name="vp", bufs=2))
        wtp = ctx.enter_context(tc.tile_pool(name="wtp", bufs=2))
        qkp = ctx.enter_context(tc.tile_pool(name="qkp", bufs=2))
        htp = ctx.enter_context(tc.tile_pool(name="htp", bufs=1))
        hnsp = ctx.enter_context(tc.tile_pool(name="hnsp", bufs=2))
        stp = ctx.enter_context(tc.tile_pool(name="stp", bufs=2))
        otp = ctx.enter_context(tc.tile_pool(name="otp", bufs=3))
        psB = ctx.enter_context(tc.tile_pool(name="psB", bufs=3, space="PSUM"))
        psS = ctx.enter_context(tc.tile_pool(name="psS", bufs=2, space="PSUM"))
        psZ = ctx.enter_context(tc.tile_pool(name="psZ", bufs=1, space="PSUM"))
        psT = ctx.enter_context(tc.tile_pool(name="psT", bufs=2, space="PSUM"))

        identB = const.tile([P, P], BF)
        make_identity(nc, identB[:])
        identF = const.tile([P, P], FP)
        make_identity(nc, identF[:])
        onesR = const.tile([P, P], FR)
        onesF = const.tile([P, P], FP)
        nc.vector.memset(onesF[:], 1.0)
        nc.vector.tensor_copy(onesR[:], onesF[:])
        eps_t = const.tile([P, 1], FP)
        nc.vector.memset(eps_t[:], EPS)
        if not g0_identity:
            g0rep = const.tile([P, E], FP)
            b0rep = const.tile([P, E], FP)
            nc.sync.dma_start(g0rep[:], g0.ap()[None, :].to_broadcast([P, E]))
            nc.sync.dma_start(b0rep[:], b0.ap()[None, :].to_broadcast([P, E]))
        if not g1_identity:
            g1rep = const.tile([P, E], FP)
            b1rep = const.tile([P, E], FP)
            nc.sync.dma_start(g1rep[:], g1.ap()[None, :].to_broadcast([P, E]))
            nc.sync.dma_start(b1rep[:], b1.ap()[None, :].to_broadcast([P, E]))

        # ---------------- stage 0: embed + LN0 + transpose -> hn0T ----------
        hn0T = hn0p.tile([P, KO, S], BF, tag="hn0")  # [e_inner, e_outer, s]
        for sb in range(SB):
            idxt = stp.tile([P, 1], mybir.dt.int32, tag="idx")
            nc.sync.dma_start(idxt[:], xidx[sb * P:(sb + 1) * P, :])
            h0sb = hnsp.tile([P, E], BF, tag="h0")
            nc.gpsimd.indirect_dma_start(
                out=h0sb[:], out_offset=None, in_=emb[:, :],
                in_offset=bass.IndirectOffsetOnAxis(ap=idxt[:, :1], axis=0),
            )
            stats = stp.tile([P, 2, 6], FP, tag="bnst")
            nc.vector.bn_stats(stats[:, 0, :], h0sb[:, 0:HALF])
            nc.vector.bn_stats(stats[:, 1, :], h0sb[:, HALF:S])
            mv = stp.tile([P, 2], FP, tag="bnmv")
            nc.vector.bn_aggr(mv[:], stats[:])
            sd = stp.tile([P, 1], FP, tag="sd")
            nc.scalar.activation(sd[:], mv[:, 1:2], AF.Sqrt, bias=eps_t[:])
            rstd = stp.tile([P, 1], FP, tag="rstd")
            nc.vector.reciprocal(rstd[:], sd[:])
            nc.vector.tensor_scalar(h0sb[:], h0sb[:], scalar1=mv[:, 0:1],
                                    scalar2=rstd[:], op0=OP.subtract,
                                    op1=OP.mult)
            if not g0_identity:
                nc.vector.tensor_tensor(h0sb[:], h0sb[:], g0rep[:], OP.mult)
                nc.vector.tensor_tensor(h0sb[:], h0sb[:], b0rep[:], OP.add)
            for eo in range(0, KO, 4):
                pst = psT.tile([P, 4, P], BF, tag="pst")
                for j in range(4):
                    nc.tensor.transpose(pst[:, j, :],
                                        h0sb[:, (eo + j) * P:(eo + j + 1) * P],
                                        identB[:])
                nc.any.tensor_copy(hn0T[:, eo:eo + 4, sb * P:(sb + 1) * P],
                                   pst[:, :, :])

        # ---------------- attention unit ------------------------------------
        def attn_unit(layer, head, hn, final):
            """hn: [P, KO, S] bf16 ([E,S] layout). If not final, writes
            ht_raw [P, SB, E] fp32 ([S,E] layout, un-normalized by z) and
            returns (ht_raw, zsq) for the fused LayerNorm; if final, scales
            by 1/z and DMAs to out_d."""
            wqk_sb = wqkp.tile([P, KO, TA], BF, tag="wqk")
            nc.sync.dma_start(wqk_sb[:],
                              wqk.ap()[layer, head].rearrange("(ko p) m -> p ko m", p=P))
            wv_sb = wvp.tile([P, KO, E], BF, tag="wv")
            nc.sync.dma_start(wv_sb[:],
                              wv.ap()[layer, head].rearrange("(ko p) o -> p ko o", p=P))

            # qkT: [2A=128, S] packed q (rows 0:64) and k (rows 64:128)
            qT = qkp.tile([A, S], BF, tag="qT")
            kT = qkp.tile([A, S], BF, tag="kT")
            for nb in range(2):
                ps_qk = psB.tile([P, HALF], FP, tag="big")
                for ko in range(KO):
                    nc.tensor.matmul(ps_qk[:],
                                     lhsT=wqk_sb[:, ko, :],
                                     rhs=hn[:, ko, nb * HALF:(nb + 1) * HALF],
                                     start=(ko == 0), stop=(ko == KO - 1))
                nc.scalar.copy(qT[:, nb * HALF:(nb + 1) * HALF], ps_qk[0:A, :])
                nc.scalar.copy(kT[:, nb * HALF:(nb + 1) * HALF], ps_qk[A:TA, :])

            # scoresT + exp -> wT [t_inner, tb, s] bf16
            wT = wtp.tile([P, SB, S], BF, tag="wT")
            for sh in range(2):
                for tb in range(SB):
                    ps_s = psS.tile([P, HALF], FP, tag="s")
                    nc.tensor.matmul(ps_s[:], lhsT=kT[:, tb * P:(tb + 1) * P],
                                     rhs=qT[:, sh * HALF:(sh + 1) * HALF],
                                     start=True, stop=True)
                    nc.scalar.activation(wT[:, tb, sh * HALF:(sh + 1) * HALF],
                                         ps_s[:], AF.Exp,
                                         scale=float(1.0 / SCALE))

            # v: [t_inner, tb, o] bf16
            v_sb = vp.tile([P, SB, E], BF, tag="v")
            for tb in range(SB):
                for nb in range(2):
                    ps_v = psB.tile([P, HALF], FP, tag="big")
                    for ko in range(KO):
                        nc.tensor.matmul(ps_v[:],
                                         lhsT=hn[:, ko, tb * P:(tb + 1) * P],
                                         rhs=wv_sb[:, ko, nb * HALF:(nb + 1) * HALF],
                                         start=(ko == 0), stop=(ko == KO - 1))
                    nc.any.tensor_copy(v_sb[:, tb, nb * HALF:(nb + 1) * HALF],
                                       ps_v[:])

            # z per half: DVE partial sums over tb, fp32r ones-matmul for the
            # cross-partition reduction, PE transpose -> per-row z columns.
            zc = stp.tile([P, SB], FP, tag="zc")  # z per s-row (col per block)
            for sh in range(2):
                zs = stp.tile([P, HALF], FR, tag="zs")
                nc.gpsimd.tensor_tensor(zs[:], wT[:, 0, sh * HALF:(sh + 1) * HALF],
                                        wT[:, 1, sh * HALF:(sh + 1) * HALF], OP.add)
                for tb in range(2, SB):
                    nc.gpsimd.tensor_tensor(zs[:], zs[:],
                                            wT[:, tb, sh * HALF:(sh + 1) * HALF],
                                            OP.add)
                ps_z = psZ.tile([P, HALF], FP, tag="z")
                nc.tensor.matmul(ps_z[:], lhsT=onesR[:],
                                 rhs=zs[:], start=True, stop=True)
                z_sb = otp.tile([P, HALF], FP, tag="zsb")
                nc.vector.tensor_copy(z_sb[:], ps_z[:])
                ps_t = psZ.tile([P, HALF], FP, tag="z")
                for sbb in range(4):
                    nc.tensor.transpose(ps_t[:, sbb * P:(sbb + 1) * P],
                                        z_sb[:, sbb * P:(sbb + 1) * P],
                                        identF[:])
                    nc.vector.tensor_copy(zc[:, sh * 4 + sbb:sh * 4 + sbb + 1],
                                          ps_t[:, sbb * P:sbb * P + 1])

            if final:
                invzc = stp.tile([P, SB], FP, tag="invzc")
                nc.vector.reciprocal(invzc[:], zc[:])
            else:
                # eps bias for the fused LN: eps * z^2 per row
                zsq = stp.tile([P, SB], FP, tag="zsq")
                nc.vector.tensor_tensor(zsq[:], zc[:], zc[:], OP.mult)
                nc.vector.tensor_scalar(zsq[:], zsq[:], scalar1=float(EPS),
                                        scalar2=None, op0=OP.mult)
                ht_raw = htp.tile([P, SB, E], FP, tag="ht")

            # out: [s_block, o] = sum_tb wT_blk^T @ v
            for blk in range(SB):
                for nb in range(2):
                    ps_o = psB.tile([P, HALF], FP, tag="big")
                    for tb in range(SB):
                        nc.tensor.matmul(ps_o[:],
                                         lhsT=wT[:, tb, blk * P:(blk + 1) * P],
                                         rhs=v_sb[:, tb, nb * HALF:(nb + 1) * HALF],
                                         start=(tb == 0), stop=(tb == SB - 1))
                    if final:
                        ot = otp.tile([P, HALF], FP, tag="ot")
                        nc.vector.tensor_scalar_mul(ot[:], ps_o[:],
                                                    invzc[:, blk:blk + 1])
                        nc.sync.dma_start(
                            out_d.ap()[head, blk * P:(blk + 1) * P,
                                       nb * HALF:(nb + 1) * HALF],
                            ot[:])
                    else:
                        nc.any.tensor_copy(
                            ht_raw[:, blk, nb * HALF:(nb + 1) * HALF], ps_o[:])

            if not final:
                return ht_raw, zsq
            return None, None

        # ---------------- fused LN between layers ---------------------------
        def ln_mid(ht_raw, zsq):
            """LayerNorm of ht_raw/z done z-free: rstd = 1/sqrt(var_raw +
            eps*z^2); then transpose into hn1T [E,S] bf16."""
            hn1T = hn1p.tile([P, KO, S], BF, tag="hn1")
            for blk in range(SB):
                stats = stp.tile([P, 2, 6], FP, tag="bnst")
                nc.vector.bn_stats(stats[:, 0, :], ht_raw[:, blk, 0:HALF])
                nc.vector.bn_stats(stats[:, 1, :], ht_raw[:, blk, HALF:E])
                mv = stp.tile([P, 2], FP, tag="bnmv")
                nc.vector.bn_aggr(mv[:], stats[:])
                sd = stp.tile([P, 1], FP, tag="sd")
                nc.scalar.activation(sd[:], mv[:, 1:2], AF.Sqrt,
                                     bias=zsq[:, blk:blk + 1])
                rstd = stp.tile([P, 1], FP, tag="rstd")
                nc.vector.reciprocal(rstd[:], sd[:])
                hb = hnsp.tile([P, E], BF, tag="hb")
                nc.vector.tensor_scalar(hb[:], ht_raw[:, blk, :],
                                        scalar1=mv[:, 0:1], scalar2=rstd[:],
                                        op0=OP.subtract, op1=OP.mult)
                if not g1_identity:
                    nc.vector.tensor_tensor(hb[:], hb[:], g1rep[:], OP.mult)
                    nc.vector.tensor_tensor(hb[:], hb[:], b1rep[:], OP.add)
                for eo in range(0, KO, 4):
                    pst = psT.tile([P, 4, P], BF, tag="pst")
                    for j in range(4):
                        nc.tensor.transpose(pst[:, j, :],
                                            hb[:, (eo + j) * P:(eo + j + 1) * P],
                                            identB[:])
                    nc.any.tensor_copy(hn1T[:, eo:eo + 4, blk * P:(blk + 1) * P],
                                       pst[:, :, :])
            return hn1T

        # ---------------- per-head: layer1 -> LN -> layer2 ------------------
        for head in range(NH):
            ht_raw, zsq = attn_unit(0, head, hn0T, final=False)
            hn1T = ln_mid(ht_raw, zsq)
            attn_unit(1, head, hn1T, final=True)

    if legalize:
        _legalize_multi_waits(nc)
    return nc


_CACHE = {}


def _get_nc(g0_identity, g1_identity, legalize=True):
    key = (g0_identity, g1_identity, legalize)
    if key not in _CACHE:
        _CACHE[key] = _build_nc(g0_identity, g1_identity, legalize)
    return _CACHE[key]


def _prep_in_maps(x, emb, ln_gamma, ln_beta, Wq, Wk, Wv):
    x = np.asarray(x)
    bf = ml_dtypes.bfloat16
    emb = np.ascontiguousarray(np.asarray(emb, dtype=np.float32).astype(bf))
    ln_gamma = np.asarray(ln_gamma, dtype=np.float32)
    ln_beta = np.asarray(ln_beta, dtype=np.float32)
    Wq = np.asarray(Wq, dtype=np.float32)
    Wk = np.asarray(Wk, dtype=np.float32)
    Wv = np.asarray(Wv, dtype=np.float32)

    # [L,H,E,2A] packed (WqT | WkT); [L,H,E,E] = WvT -- bf16
    wqkT = np.concatenate([Wq.transpose(0, 1, 3, 2), Wk.transpose(0, 1, 3, 2)],
                          axis=3).astype(bf)
    wvT = Wv.transpose(0, 1, 3, 2).astype(bf)

    in_maps = []
    for c in range(8):
        b = c // 2
        hs = (c % 2) * NH
        in_maps.append({
            "emb": emb,
            "xidx": np.ascontiguousarray(x[b].astype(np.int32).reshape(S, 1)),
            "wqk": np.ascontiguousarray(wqkT[:, hs:hs + NH]),
            "wv": np.ascontiguousarray(wvT[:, hs:hs + NH]),
            "g0": np.ascontiguousarray(ln_gamma[0]),
            "b0": np.ascontiguousarray(ln_beta[0]),
            "g1": np.ascontiguousarray(ln_gamma[1]),
            "b1": np.ascontiguousarray(ln_beta[1]),
        })
    g0_id = bool(np.all(ln_gamma[0] == 1.0) and np.all(ln_beta[0] == 0.0))
    g1_id = bool(np.all(ln_gamma[1] == 1.0) and np.all(ln_beta[1] == 0.0))
    return in_maps, g0_id, g1_id


def run(inputs, trace=False, trace_cores=None):
    in_maps, g0_id, g1_id = _prep_in_maps(**inputs)
    nc = _get_nc(g0_id, g1_id)
    res = run_bass_kernel_spmd(nc, in_maps, core_ids=list(range(8)),
                               trace=trace, trace_cores=trace_cores)
    out = np.empty((B, H, S, E), dtype=np.float32)
    for c in range(8):
        out[c // 2, (c % 2) * NH:(c % 2) * NH + NH] = res.results[c]["out"]
    return out, res


def kernel(x, emb, ln_gamma, ln_beta, Wq, Wk, Wv):
    out, _ = run(dict(x=x, emb=emb, ln_gamma=ln_gamma, ln_beta=ln_beta,
                      Wq=Wq, Wk=Wk, Wv=Wv))
    return out
